# revision 36
# baseline (speedup 1.0000x reference)
"""Trainium2 Bass kernel for a 2-head MultiHeadAttn + residual + LayerNorm block.

Problem shapes (hardcoded):
  x:      [8, 2048, 384] f32      attn_mask: [8, 2048] bool (True = attend)
  qkv_w:  [384, 384] f32          qkv_b: [384] f32
  o_w:    [128, 384] f32          ln_g, ln_b: [384] f32
  out:    [8, 2048, 384] f32

Sharding: data-parallel over batch - 8 batch elements, one per NeuronCore.

Per-core dataflow (S=2048, D_model=384, H=2, Dh=64), all on-chip.  The
steady-state loop is gated by the ACT-engine exp (1 elem/lane/cycle @
1.2 GHz -> ~1 us per 128x1024 chunk, 64 chunks); everything else hides
under it:
  - scores: bf16, both heads as row-tiled concurrent K=64 matmuls
    (h0 rows 0-63, h1 rows 64-127) -> psum [128, 2, 512]
  - exp: one ACT instr per chunk, PSUM fp32 -> SBUF fp8e4 (probs are
    ~1/2048-weight averages downstream; fp8 noise is ~1e-4 of output)
  - pv: fp8 DoubleRow over chunk PAIRS (virtual K=256): per pair and
    head one DR matmul, lhsT = vt8 [128, 2, 65] (V columns + a ones
    column -> psum partition 64 accumulates the softmax denominator),
    rhs = exd8 [128, 2, 512]
  - quarter tail: ONE DVE drain copy [65, 2, 512] (raw attn + denom
    rows), K=1 ones(1/16)-matmul broadcasts denom/16 -> reciprocal ->
    16/denom; normalize-mul writes attnT8 (fp8, x16 so values sit in
    e4m3's normal range)
  - per chunk: o-proj = ONE fp8 DR matmul (heads are the two k-tiles)
    + a 16*identity matmul folds in the residual -> po = 16*(x+attn);
    LayerNorm is scale-invariant so stats/normalize run on 16y with
    eps*256; rstd via DVE magic-rsqrt (no second ACT table)
  - preamble: x^T transposes (identity matmuls, drain copies split
    ACT/DVE) + ALL K strips run DMA-paced up front - weights consumed
    as matmul lhsT must be produced long before their LDWEIGHTS enters
    the PE queue (walrus sometimes elides the LDW wait and the PE
    reorder window hoists it past stalled instructions into unwritten
    SBUF).  V chunks + Q strips (moving-operand consumers, safe) drain
    as in-loop filler.  Quarter-norm broadcasts land in po-pool banks,
    never the scores/pv rings (a WAR on the reciprocal there stalls
    exp ~1-2us per quarter).  The last quarter's tail runs post-loop
    with 4-wide psum ILP and the normalize split across idle ACT
    (Copy with per-partition rstd scale) and DVE (mean subtract).
"""

import os
import sys

import ml_dtypes
import numpy as np

for _p in ("/opt/trn_rl_repo", "/root/.axon_site/_ro/trn_rl_repo"):
    if os.path.isdir(_p) and _p not in sys.path:
        sys.path.insert(0, _p)

import concourse.bass as bass  # noqa: E402
import concourse.tile as tile  # noqa: E402
from concourse import bacc  # noqa: E402
from concourse import mybir  # noqa: E402
from concourse.bass_utils import run_bass_kernel_spmd  # noqa: E402
from concourse.masks import make_identity  # noqa: E402

FP = mybir.dt.float32
BF = mybir.dt.bfloat16
F8 = mybir.dt.float8e4
I32 = mybir.dt.int32
AF = mybir.ActivationFunctionType
OP = mybir.AluOpType
DR = mybir.MatmulPerfMode.DoubleRow

B, S, DM = 8, 2048, 384
H, DH = 2, 64
INNER = H * DH  # 128
P = 128
SC = S // P  # 16 k-chunks of 128
NPAIR = SC // 2  # 8 fp8 DoubleRow k-pairs
DC = DM // P  # 3 model-dim chunks of 128
NQ = 4  # q-quarters
QW = S // NQ  # 512
LN_EPS = 1e-3
N_CORES = 8
SCALE = 1.0 / (DH**0.5)
RS = 16.0  # attnT/residual up-scale (LayerNorm is scale-invariant)
# Schraudolph exp-to-fp8e4: bits = round(s*log2(e)*8*SCALE + 56 - 8c).
# With SCALE = 1/8 the slope is exactly log2(e).  Systematic bias
# cancels in the softmax ratio; residual spread ~3% = plain-fp8 level.
EXP_A = 1.4426950408889634
EXP_B = 56.0 - 8 * 0.043
# steps whose exp runs on the DVE instead of ACT (one tensor_scalar
# writing the fp8 bit pattern as int8).  Each offloaded chunk saves a
# full ~1038ns ACT slot minus a ~400ns scores bubble (the scores WAR
# partner runs after the preceding ACT exp instead of under it).
# Avoid quarter 0 (DVE busy with projection filler) and keep offloads
# >=2 chunks apart so the ping-pong WAR chain never doubles up.
DVE_EXP = {(qq, c) for qq in (1, 2, 3) for c in (4, 7, 10, 13)}


def _build(has_mask: bool, has_bias: bool, has_affine: bool) -> bass.Bass:
    nc = bacc.Bacc(
        "TRN2", target_bir_lowering=False, debug=False, num_devices=N_CORES
    )

    xb_d = nc.dram_tensor("x_bf", [S, DM], BF, kind="ExternalInput")
    w_d = nc.dram_tensor("qkv_w_bf", [DM, 3 * INNER], BF, kind="ExternalInput")
    ow_d = nc.dram_tensor("o_w_bf", [INNER, DM], BF, kind="ExternalInput")
    mask_d = bias_d = g_d = b_d = None
    if has_mask:
        mask_d = nc.dram_tensor("mask_f", [S], FP, kind="ExternalInput")
    if has_bias:
        bias_d = nc.dram_tensor("qkv_b", [3 * INNER], FP, kind="ExternalInput")
    if has_affine:
        g_d = nc.dram_tensor("ln_g", [DM], FP, kind="ExternalInput")
        b_d = nc.dram_tensor("ln_b", [DM], FP, kind="ExternalInput")
    y_d = nc.dram_tensor("y", [S, DM], FP, kind="ExternalOutput")
    debug = os.environ.get("KBENCH_DEBUG", "0") == "1"
    dbg = {}
    if debug:
        dbg["qkvT"] = nc.dram_tensor("dbg_qkvT", [P, 2, S], FP, kind="ExternalOutput")
        dbg["vt8"] = nc.dram_tensor(
            "dbg_vt8", [P, NPAIR, 2, H, 80], FP, kind="ExternalOutput"
        )
        dbg["exd8"] = nc.dram_tensor(
            "dbg_exd8", [P, 3, 2, H, QW], FP, kind="ExternalOutput"
        )
        dbg["attnT8"] = nc.dram_tensor(
            "dbg_attnT8", [DH, H, S], FP, kind="ExternalOutput"
        )
        dbg["mv"] = nc.dram_tensor("dbg_mv", [P, SC, 2], FP, kind="ExternalOutput")
        dbg["rstd"] = nc.dram_tensor("dbg_rstd", [P, SC], FP, kind="ExternalOutput")
        dbg["rb"] = nc.dram_tensor("dbg_rb", [DH, H, QW], FP, kind="ExternalOutput")
        dbg["xT"] = nc.dram_tensor("dbg_xT", [P, DC, S], FP, kind="ExternalOutput")

    with tile.TileContext(nc) as tc:
        with tc.tile_pool(name="singles", bufs=1) as sg:
            # ---- constants / warmup ----
            wu = sg.tile([P, 512], BF, tag="wu")
            nc.vector.memset(wu, 0.125)
            # trigger the exp table-set DMA (~2.7us) at kernel start,
            # long before the first real exp
            exp_warm = sg.tile([1, 4], FP, tag="exp_warm")
            nc.scalar.activation(exp_warm, wu[0:1, 0:4], AF.Exp, scale=SCALE)

            ident = sg.tile([P, P], BF, tag="ident")
            make_identity(nc, ident)
            ident16 = sg.tile([P, P], BF, tag="ident16")
            nc.vector.tensor_scalar(
                ident16, ident, scalar1=RS, scalar2=None, op0=OP.mult
            )

            # ---- input DMAs: first 4 x-chunks first, weights next ----
            x_sb = sg.tile([P, SC, DM], BF, tag="x_sb")
            x_r = xb_d.rearrange("(c p) d -> p c d", p=P)
            w_sb = sg.tile([P, DC, 3 * INNER], BF, tag="w_sb")
            ow_sb = sg.tile([DH, H, DM], BF, tag="ow_sb")
            for c in range(4):
                eng = nc.sync if c % 2 == 0 else nc.scalar
                eng.dma_start(x_sb[:, c, :], x_r[:, c, :])
            nc.sync.dma_start(w_sb, w_d.rearrange("(dc dp) j -> dp dc j", dp=P))
            nc.scalar.dma_start(ow_sb, ow_d.rearrange("(h d) m -> d h m", d=DH))
            for c in range(4, SC):
                eng = nc.sync if c % 2 == 0 else nc.scalar
                eng.dma_start(x_sb[:, c, :], x_r[:, c, :])

            ow8 = sg.tile([DH, H, DM], F8, tag="ow8")
            nc.vector.tensor_copy(ow8, ow_sb)

            xT = sg.tile([P, DC, S], BF, tag="xT")
            qkvT = sg.tile([P, 2, S], BF, tag="qkvT")  # 0=Q^T 1=K^T

            eps_sb = sg.tile([P, 1], FP, tag="eps")
            nc.vector.memset(eps_sb, LN_EPS * RS * RS)

            mask_sb = bias_sb = g_sb = b_sb = bco_sb = ow2_sb = None
            if mask_d is not None:
                mask_sb = sg.tile([P, SC], FP, tag="mask_sb")
                nc.sync.dma_start(mask_sb, mask_d.rearrange("(c p) -> p c", p=P))
            if bias_d is not None:
                bias_sb = sg.tile([P, 3], FP, tag="bias_sb")
                nc.sync.dma_start(bias_sb, bias_d.rearrange("(jt p) -> p jt", p=P))
                ow2_sb = sg.tile([INNER, DM], BF, tag="ow2_sb")
                nc.sync.dma_start(ow2_sb, ow_d)
            if g_d is not None and b_d is not None:
                g_sb = sg.tile([P, DM], FP, tag="g_sb")
                b_sb = sg.tile([P, DM], FP, tag="b_sb")
                nc.gpsimd.dma_start(g_sb, g_d[None, :].to_broadcast((P, DM)))
                nc.gpsimd.dma_start(b_sb, b_d[None, :].to_broadcast((P, DM)))

            # vt8: V in fp8 DoubleRow layout [k, pair, ko, h, 80]:
            # cols 0-63 = V, col 64 = ones (-> psum row 64 = denominator)
            vt8 = sg.tile([P, NPAIR, 2, H, 80], F8, tag="vt8")
            if mask_sb is not None:
                mr = mask_sb.rearrange("p (pr ko) -> p pr ko", ko=2)
                for h in range(H):
                    nc.vector.tensor_copy(vt8[:, :, :, h, 64:65], mr[:, :, :, None])
            else:
                nc.vector.memset(vt8[:, :, :, :, 64:65], 1.0)

            # exp output ring: 3 pairs deep [k, pair%3, ko, h, q]
            # (h, q innermost so the ACT exp writes one CONTIGUOUS
            # [128, 1024] fp8 block; the DR rhs view strides over ko)
            exd8 = sg.tile([P, 3, 2, H, QW], F8, tag="exd8")
            # normalized attn^T, fp8, x16: [d, h, s]
            attnT8 = sg.tile([DH, H, S], F8, tag="attnT8")
            # ones(1/16) rows: broadcast-matmul weights for denom/16.
            # Partition 64 copy matches ar2's denominator-row base
            # partition (matmul requires lhsT/rhs base partitions equal).
            ones16 = sg.tile([65, P], BF, tag="ones16")
            nc.vector.memset(ones16, 1.0 / RS)
            rb = sg.tile([DH, H, QW], FP, tag="rb")  # 16/denom
            mv_sb = sg.tile([P, SC, 2], FP, tag="mv_sb")
            rstd_sb = sg.tile([P, SC], FP, tag="rstd_sb")
            magic_sb = sg.tile([P, 4], I32, tag="magic_sb")
            nc.vector.memset(magic_sb, 0x5F3759DF)
            ve_sb = sg.tile([P, SC], FP, tag="ve_sb")
            ti_sb = sg.tile([P, SC], I32, tag="ti_sb")
            t1_sb = sg.tile([P, SC], FP, tag="t1_sb")
            t2_sb = sg.tile([P, SC], FP, tag="t2_sb")
            o_sb = sg.tile([P, SC, DM], FP, tag="o_sb")

            with (
                tc.tile_pool(name="ps_sc", bufs=2, space="PSUM") as sc_pool,
                tc.tile_pool(name="ps_pv", bufs=1, space="PSUM") as pv_pool,
                tc.tile_pool(name="ps_po", bufs=2, space="PSUM") as po_pool,
                tc.tile_pool(name="post", bufs=4) as post,
            ):
                y_t3 = y_d.rearrange("(c p) m -> p c m", p=P)

                def po_tile(name):
                    return po_pool.tile([P, 512], FP, tag="po", name=name)

                def sc_tile(name):
                    t = sc_pool.tile([P, H, QW], FP, tag="sc", name=name)
                    return t.rearrange("p a b -> p (a b)")

                # -- projection/transpose emitters --
                def xt_block(c, mk=po_tile):
                    tp = mk(f"xt{c}")[:, 0:512]
                    for dc in range(DC):
                        nc.tensor.matmul(
                            tp[:, dc * P : (dc + 1) * P],
                            lhsT=x_sb[:, c, dc * P : (dc + 1) * P],
                            rhs=ident,
                            start=True,
                            stop=True,
                        )
                    # drain copies alternate ACT/DVE - the preamble is
                    # otherwise paced by 16 serial DVE casts while the
                    # scalar engine idles behind the exp table load
                    eng = nc.scalar if c % 2 == 0 else nc.vector
                    (eng.copy if eng is nc.scalar else nc.vector.tensor_copy)(
                        xT[:, :, c * P : (c + 1) * P],
                        tp[:, 0:DM].rearrange("p (dc q) -> p dc q", dc=DC),
                    )

                def qk_mms(jt, st, mk=po_tile):
                    pq = mk(f"qk{jt}_{st}")[:, 0:512]
                    for dc in range(DC):
                        nc.tensor.matmul(
                            pq,
                            lhsT=w_sb[:, dc, jt * P : (jt + 1) * P],
                            rhs=xT[:, dc, st * 512 : (st + 1) * 512],
                            start=(dc == 0),
                            stop=(dc == DC - 1),
                        )
                    dst = qkvT[:, jt, st * 512 : (st + 1) * 512]
                    if bias_sb is not None:
                        nc.vector.tensor_scalar_add(dst, pq, bias_sb[:, jt : jt + 1])
                    else:
                        nc.vector.tensor_copy(dst, pq)

                def v_mms(c, mk=po_tile):
                    vp = mk(f"vp{c}")
                    for dc in range(DC):
                        nc.tensor.matmul(
                            vp[:, 0:P],
                            lhsT=xT[:, dc, c * P : (c + 1) * P],
                            rhs=w_sb[:, dc, 2 * P : 3 * P],
                            start=(dc == 0),
                            stop=(dc == DC - 1),
                        )
                    # psum [128 k, (h d)] -> vt8[k, pair, ko, h, 0:64]
                    dst = vt8[:, c // 2, c % 2, :, 0:DH]
                    src = vp[:, 0:P].rearrange("p (h d) -> p h d", h=H)
                    if mask_sb is not None:
                        nc.vector.tensor_scalar_mul(
                            dst, src, mask_sb[:, c : c + 1]
                        )
                    else:
                        nc.vector.tensor_copy(dst, src)

                # V-bias would need the baseline's bco fold; the bias
                # variant keeps it via a y-add below
                if bias_sb is not None:
                    bvec_bf = sg.tile([P, 1], BF, tag="bvec_bf")
                    nc.vector.tensor_copy(bvec_bf, bias_sb[:, 2:3])
                    pbv = po_tile("pbv")
                    nc.tensor.matmul(
                        pbv[0:1, 0:DM], lhsT=bvec_bf, rhs=ow2_sb,
                        start=True, stop=True,
                    )
                    bvo_row = sg.tile([1, DM], BF, tag="bvo_row")
                    nc.vector.tensor_copy(bvo_row, pbv[0:1, 0:DM])
                    pbc = po_tile("pbc")
                    nc.tensor.matmul(
                        pbc[:, 0:DM],
                        lhsT=ones16[0:1, :],
                        rhs=bvo_row,
                        start=True,
                        stop=True,
                    )
                    # bco = RS^2 * (b_v @ o_w) broadcast (ones16 = 1/16,
                    # compensate with RS^2 so the po-space add is RS*bvo)
                    bco_sb = sg.tile([P, DM], FP, tag="bco_sb")
                    nc.vector.tensor_scalar(
                        bco_sb, pbc[:, 0:DM],
                        scalar1=RS * RS, scalar2=None, op0=OP.mult,
                    )

                # ---- PE warm-up + preamble ----
                # Weights-hazard discipline: anything consumed as a
                # matmul lhsT (K strips, xT for the V projections) is
                # produced either here in the DMA-paced preamble or, for
                # vt8/attnT8, >=4 steps before first consumption - the
                # PE reorder window can hoist a waitless LDWEIGHTS past
                # stalled instructions and read SBUF before the
                # producing DVE copy lands.
                wps = po_tile("warm")
                for _ in range(2):
                    nc.tensor.matmul(
                        wps, lhsT=wu[:, 0:P], rhs=wu, start=True, stop=True
                    )
                for c in range(SC):
                    xt_block(c, mk=(po_tile if c % 2 == 0 else sc_tile))
                    if c % 4 == 3:
                        # K strip as soon as its xt quad lands
                        qk_mms(1, c // 4, mk=sc_tile)
                qk_mms(0, 0, mk=sc_tile)
                v_mms(0)
                v_mms(1)

                # in-loop filler: V chunks (xT is preamble-old), then Q
                # strips (consumed as the scores MOVING operand - safe)
                prework = [lambda c=c: v_mms(c) for c in range(2, SC)]
                prework += [lambda: qk_mms(0, 1)]
                prework += [lambda: qk_mms(0, 2)]
                prework += [lambda: qk_mms(0, 3)]

                # ---- attention loop emitters ----
                def emit_scores(qq, c):
                    ps = sc_pool.tile([P, H, QW], FP, tag="sc", name=f"sc{qq}_{c}")
                    for h in range(H):
                        hs = slice(h * DH, (h + 1) * DH)
                        nc.tensor.matmul(
                            ps[:, h, :],
                            lhsT=qkvT[hs, 1, c * P : (c + 1) * P],
                            rhs=qkvT[hs, 0, qq * QW : (qq + 1) * QW],
                            start=True,
                            stop=True,
                        )
                    return ps

                def emit_exp(qq, c, ps):
                    dst = exd8[:, (c // 2) % 3, c % 2, :, :]
                    if (qq, c) in DVE_EXP:
                        nc.vector.tensor_scalar(
                            dst.bitcast(mybir.dt.int8),
                            ps,
                            scalar1=EXP_A,
                            scalar2=EXP_B,
                            op0=OP.mult,
                            op1=OP.add,
                        )
                    else:
                        nc.scalar.activation(dst, ps, AF.Exp, scale=SCALE)

                ppv = [None]

                def emit_pv(qq, pr, h):
                    if pr == 0 and h == 0:
                        ppv[0] = pv_pool.tile(
                            [65, H, QW], FP, tag="pv", name=f"pv{qq}"
                        )
                    nc.tensor.matmul(
                        ppv[0][:, h, :],
                        lhsT=vt8[:, pr, :, h, 0:65],
                        rhs=exd8[:, pr % 3, :, h, :],
                        start=(pr == 0),
                        stop=(pr == NPAIR - 1),
                        perf_mode=DR,
                    )

                def emit_drain(qq):
                    # one copy: V rows 0-63 + denominator row 64.  The
                    # final quarter drains post-loop when ACT is idle -
                    # ScalarE reads PSUM faster and unblocks the DVE
                    # tail chain.
                    ar2 = post.tile([65, H, QW], BF, tag="ar2", name=f"ar2_{qq}")
                    if qq == NQ - 1:
                        nc.scalar.copy(ar2, ppv[0])
                    else:
                        nc.vector.tensor_copy(ar2, ppv[0])
                    return ar2

                def emit_norm_bcast(qq, ar2):
                    # broadcast denom/16 into two po-pool banks (NOT the
                    # scores ring - that hands the next scores allocation
                    # a WAR on the reciprocal; NOT the pv psum - that
                    # stalls the next quarter's first pv pair ~1.2us
                    # behind the reciprocal read)
                    pbs = []
                    for h in range(H):
                        pb = po_tile(f"pb{qq}_{h}")
                        nc.tensor.matmul(
                            pb[0:DH, 0:QW],
                            lhsT=ones16[64:65, 0:DH],
                            rhs=ar2[64:65, h, :],
                            start=True,
                            stop=True,
                        )
                        pbs.append(pb)
                    for h in range(H):
                        nc.vector.reciprocal_approx_fast(
                            rb[:, h, :], pbs[h][0:DH, 0:QW]
                        )

                def emit_norm_mul(qq, ar2, h):
                    q0 = qq * QW
                    nc.vector.tensor_mul(
                        attnT8[:, h, q0 : q0 + QW], ar2[0:DH, h, :], rb[:, h, :]
                    )

                def emit_tail_chunk(ch, po=None):
                    if po is None:
                        po = po_tile(f"po{ch}")
                    nc.tensor.matmul(
                        po[:, 0:DM],
                        lhsT=ident16,
                        rhs=x_sb[:, ch, :],
                        start=True,
                        stop=False,
                    )
                    nc.tensor.matmul(
                        po[:, 0:DM],
                        lhsT=attnT8[:, :, ch * P : (ch + 1) * P],
                        rhs=ow8,
                        start=False,
                        stop=True,
                        perf_mode=DR,
                    )
                    if bco_sb is not None:
                        nc.vector.tensor_add(po[:, 0:DM], po[:, 0:DM], bco_sb)
                    st6 = post.tile([P, 6], FP, tag="st6")
                    nc.vector.bn_stats(st6, po[:, 0:DM])
                    nc.vector.bn_aggr(mv_sb[:, ch, :], st6)
                    return po

                def emit_rstd2(ch0):
                    # rstd = (var+eps)^-1/2 via magic-seed + 1 Newton
                    # iteration, two chunks at a time (var is of 16y;
                    # eps is pre-scaled by 256)
                    s = slice(ch0, ch0 + 2)
                    ve, ti = ve_sb[:, s], ti_sb[:, s]
                    t1, t2 = t1_sb[:, s], t2_sb[:, s]
                    nc.vector.tensor_scalar_add(ve, mv_sb[:, s, 1], eps_sb)
                    nc.vector.tensor_scalar(
                        ti,
                        ve.bitcast(I32),
                        scalar1=1,
                        scalar2=None,
                        op0=OP.logical_shift_right,
                    )
                    nc.vector.tensor_sub(t1.bitcast(I32), magic_sb[:, 0:2], ti)
                    nc.vector.tensor_mul(t2, t1, t1)
                    nc.vector.tensor_mul(t2, t2, ve)
                    nc.vector.tensor_scalar(
                        t2, t2, scalar1=-0.5, scalar2=1.5,
                        op0=OP.mult, op1=OP.add,
                    )
                    nc.vector.tensor_mul(rstd_sb[:, s], t1, t2)

                def emit_ts_store(ch, po):
                    o_t = o_sb[:, ch, :]
                    nc.vector.tensor_scalar(
                        o_t,
                        po[:, 0:DM],
                        scalar1=mv_sb[:, ch, 0:1],
                        scalar2=rstd_sb[:, ch : ch + 1],
                        op0=OP.subtract,
                        op1=OP.mult,
                    )
                    if g_sb is not None and b_sb is not None:
                        nc.vector.tensor_mul(o_t, o_t, g_sb)
                        nc.vector.tensor_add(o_t, o_t, b_sb)
                    if ch % 2 == 1:
                        eng = nc.gpsimd if ch % 4 == 1 else nc.sync
                        eng.dma_start(
                            y_t3[:, ch - 1 : ch + 1, :],
                            o_sb[:, ch - 1 : ch + 1, :],
                        )

                # ---- flat pipelined emission over 64 (qq, c) steps ----
                # pv runs one DR matmul (one head) per step, lagging exp
                # by ~2 chunks; tailwork entries carry an earliest-step
                # gate so o-proj consumes attnT8 >=4 steps after the
                # normalize muls wrote it
                pend_pv = []
                tailwork = []  # (earliest_step, fn)
                step = [0]

                def emit_rstd4(c0):
                    s = slice(c0, c0 + 4)
                    ve, ti = ve_sb[:, s], ti_sb[:, s]
                    t1, t2 = t1_sb[:, s], t2_sb[:, s]
                    nc.vector.tensor_scalar_add(ve, mv_sb[:, s, 1], eps_sb)
                    nc.vector.tensor_scalar(
                        ti, ve.bitcast(I32),
                        scalar1=1, scalar2=None, op0=OP.logical_shift_right,
                    )
                    nc.vector.tensor_sub(t1.bitcast(I32), magic_sb[:, 0:4], ti)
                    nc.vector.tensor_mul(t2, t1, t1)
                    nc.vector.tensor_mul(t2, t2, ve)
                    nc.vector.tensor_scalar(
                        t2, t2, scalar1=-0.5, scalar2=1.5,
                        op0=OP.mult, op1=OP.add,
                    )
                    nc.vector.tensor_mul(rstd_sb[:, s], t1, t2)

                def final_tail(pqq, ar2):
                    # last quarter runs post-loop: maximize ILP instead
                    # of the in-loop pairwise po-ring order - 4 psum
                    # slots carved from the idle scores pool, then
                    # batched bn / rstd / normalize chains
                    emit_norm_bcast(pqq, ar2)
                    for h in range(H):
                        emit_norm_mul(pqq, ar2, h)
                    c0 = pqq * 4
                    slots = []
                    for i in range(2):
                        t = sc_pool.tile([P, H, QW], FP, tag="sc", name=f"ft{i}")
                        slots += [t[:, 0, :], t[:, 1, :]]
                    pos = [
                        emit_tail_chunk(c0 + i, po=slots[i]) for i in range(4)
                    ]
                    emit_rstd4(c0)
                    if g_sb is not None:
                        for i in range(4):
                            emit_ts_store(c0 + i, pos[i])
                        return
                    # normalize split across the idle ACT and the DVE:
                    # o = po*rstd (ScalarE Copy, per-partition scale)
                    #   - mean*rstd (DVE per-partition subtract)
                    s = slice(c0, c0 + 4)
                    nc.vector.tensor_mul(
                        ve_sb[:, s], mv_sb[:, s, 0], rstd_sb[:, s]
                    )
                    for i in range(4):
                        ch = c0 + i
                        o_t = o_sb[:, ch, :]
                        nc.scalar.activation(
                            o_t,
                            pos[i][:, 0:DM],
                            AF.Copy,
                            scale=rstd_sb[:, ch : ch + 1],
                        )
                        nc.vector.tensor_scalar_sub(
                            o_t, o_t, ve_sb[:, ch : ch + 1]
                        )
                        if ch % 2 == 1:
                            eng = nc.gpsimd if ch % 4 == 1 else nc.sync
                            eng.dma_start(
                                y_t3[:, ch - 1 : ch + 1, :],
                                o_sb[:, ch - 1 : ch + 1, :],
                            )

                def queue_tail(pqq, box):
                    g0 = step[0]
                    c0 = pqq * 4
                    pos = [None, None, None, None]

                    def tl(i):
                        pos[i] = emit_tail_chunk(c0 + i)

                    tailwork.append((g0 + 1, lambda: emit_norm_bcast(pqq, box[0])))
                    tailwork.append((g0 + 2, lambda: emit_norm_mul(pqq, box[0], 0)))
                    tailwork.append((g0 + 2, lambda: emit_norm_mul(pqq, box[0], 1)))
                    tailwork.append((g0 + 6, lambda: tl(0)))
                    tailwork.append((g0 + 7, lambda: tl(1)))
                    tailwork.append((g0 + 8, lambda: emit_rstd2(c0)))
                    tailwork.append((g0 + 8, lambda: emit_ts_store(c0, pos[0])))
                    tailwork.append((g0 + 9, lambda: emit_ts_store(c0 + 1, pos[1])))
                    tailwork.append((g0 + 10, lambda: tl(2)))
                    tailwork.append((g0 + 11, lambda: tl(3)))
                    tailwork.append((g0 + 12, lambda: emit_rstd2(c0 + 2)))
                    tailwork.append((g0 + 12, lambda: emit_ts_store(c0 + 2, pos[2])))
                    tailwork.append((g0 + 13, lambda: emit_ts_store(c0 + 3, pos[3])))

                def pop_pv():
                    pqq, pr, h = pend_pv.pop(0)
                    emit_pv(pqq, pr, h)
                    if pr == NPAIR - 1 and h == H - 1:
                        ar2 = emit_drain(pqq)
                        if pqq == NQ - 1:
                            tailwork.append((0, lambda: final_tail(pqq, ar2)))
                        else:
                            queue_tail(pqq, [ar2])

                if os.environ.get("KBENCH_PRE", "0") == "1":
                    while prework:
                        prework.pop(0)()

                for g in range(NQ * SC):
                    step[0] = g
                    qq, c = divmod(g, SC)
                    ps = emit_scores(qq, c)
                    emit_exp(qq, c, ps)
                    if c % 2 == 1:
                        pend_pv.append((qq, c // 2, 0))
                        pend_pv.append((qq, c // 2, 1))
                    if c == 0 and g > 0:
                        # flush the previous quarter's pv backlog right
                        # after this step's scores: their exps are done,
                        # and the drain frees the pv psum well before
                        # this quarter's first pv pair needs it
                        while pend_pv and pend_pv[0][0] == qq - 1:
                            pop_pv()
                    elif len(pend_pv) > 2:
                        pop_pv()
                    npop = 0
                    while prework and npop < 2:
                        prework.pop(0)()
                        npop += 1
                    if (
                        not prework
                        and npop < 2
                        and tailwork
                        and tailwork[0][0] <= g
                    ):
                        tailwork.pop(0)[1]()
                step[0] = NQ * SC + 100  # release all earliest-step gates
                while pend_pv:
                    pop_pv()
                while tailwork:
                    tailwork.pop(0)[1]()

    nc.compile()
    return nc


_PROGRAM_CACHE: dict = {}


def _get_program(key):
    if key not in _PROGRAM_CACHE:
        _PROGRAM_CACHE[key] = _build(*key)
    return _PROGRAM_CACHE[key]


def kernel(x, attn_mask, qkv_w, qkv_b, o_w, ln_g, ln_b, **_ignored):
    x = np.ascontiguousarray(np.asarray(x, dtype=np.float32))
    attn_mask = np.asarray(attn_mask)
    qkv_w = np.ascontiguousarray(np.asarray(qkv_w, dtype=np.float32))
    qkv_b = np.asarray(qkv_b, dtype=np.float32)
    o_w = np.ascontiguousarray(np.asarray(o_w, dtype=np.float32))
    ln_g = np.asarray(ln_g, dtype=np.float32)
    ln_b = np.asarray(ln_b, dtype=np.float32)

    has_mask = not bool(attn_mask.all())
    has_bias = bool(np.any(qkv_b != 0.0))
    has_affine = bool(np.any(ln_g != 1.0) or np.any(ln_b != 0.0))

    nc = _get_program((has_mask, has_bias, has_affine))

    mask_f = attn_mask.astype(np.float32)
    in_maps = []
    for i in range(N_CORES):
        m = {
            "x_bf": np.ascontiguousarray(x[i].astype(ml_dtypes.bfloat16)),
            "qkv_w_bf": qkv_w.astype(ml_dtypes.bfloat16),
            "o_w_bf": o_w.astype(ml_dtypes.bfloat16),
        }
        if has_mask:
            m["mask_f"] = np.ascontiguousarray(mask_f[i])
        if has_bias:
            m["qkv_b"] = qkv_b
        if has_affine:
            m["ln_g"] = ln_g
            m["ln_b"] = ln_b
        in_maps.append(m)

    trace = os.environ.get("KBENCH_TRACE", "0") == "1"
    kw = {}
    if trace:
        kw = {"trace": True, "trace_cores": [0]}
    res = run_bass_kernel_spmd(nc, in_maps, core_ids=list(range(N_CORES)), **kw)
    global LAST_RESULT
    LAST_RESULT = res
    return np.stack([res.results[i]["y"] for i in range(N_CORES)], axis=0)


LAST_RESULT = None
